# revision 9
# baseline (speedup 1.0000x reference)
"""Multi-head attention on 8 TRN2 NeuronCores.

Problem: x[2, 2048, 1024] @ w_qkv[1024, 3072] -> 16-head attention -> @ w_o[1024, 1024].

Sharding: core c handles batch b = c//4 and 4 heads [4*(c%4), 4*(c%4)+4).
Each core computes a full partial output y_c[2048, 1024] = attn_out_heads @ w_o_rows;
host sums the 4 partials per batch (the "all-reduce" of the row-split w_o).

Per-core layouts (host-prepped so the kernel never transposes on-chip):
  xT  [1024, 2048]  = x[b].T
  wqk [1024, 512]   cols = q(h0),q(h1),q(h2),q(h3),k(h0)..k(h3)  (64 each)
  wv  [1024, 256]   cols = v(h0)..v(h3)
  wo  [256, 1024]   rows = w_o rows for the 4 heads
Phase 1 (projections):
  qT/kT via weights-stationary matmuls -> [head-dim rows, tokens] directly
  V    via xT-stationary matmuls       -> [tokens, head cols] directly
Phase 2 (attention, per 2-head group, per 512-query chunk, streaming over 16 key tiles):
  scoresT[t,s] psum = kT.T @ qT   (two K=64 matmuls row-packed at tile_position (0,0)/(64,0))
  expT = exp(scoresT * 0.125) on ACT (scores ~ N(0,1): no max subtraction needed)
  [uout | den] += [V_tile | ones].T @ expT   (M=65 matmul per head: rows 0-63 are
      the attention output, row 64 is the softmax denominator; fp32r cannot
      col-tile on this walrus, so the two heads are sequential streams)
  normalize: reciprocal(den) -> DMA partition-broadcast -> multiply -> outT_h[d,s]
Phase 3: y[s, :] = sum_h outT_h.T @ wo_h (K=64 per head), DMA via SBUF to DRAM.
"""

import os
from contextlib import ExitStack

import numpy as np

import concourse.bass as bass
import concourse.tile as tile
from concourse import bacc, mybir
from concourse.bass_utils import run_bass_kernel_spmd

F32 = mybir.dt.float32
# float32r: full-rate (1 cyc/row) PE matmuls on fp32 data with slightly
# reduced multiply precision (the BIR verifier requires every fp32r matmul
# operand to be *produced* as fp32r, so the whole data chain up to the PE is
# declared float32r; its numpy binding is plain float32). Set BASS_MHA_FP32=1
# to fall back to exact (4x slower) fp32 matmuls.
MM_DT = F32 if os.environ.get("BASS_MHA_FP32") == "1" else mybir.dt.float32r

B, S, D = 2, 2048, 1024
H, DK = 16, 64
N_CORES = 8
HPC = 4           # heads per core
NGROUPS = 2       # head groups per core (2 heads each)
SC = 512          # query-chunk (matmul streaming N)
NSC = S // SC     # 4 query chunks
NT = S // 128     # 16 key tiles
NCH = D // 128    # 8 contraction tiles for the projections
SCALE = DK ** -0.5


def _mm(nc, out, lhsT, rhs, **kw):
    nc.tensor.matmul(out, lhsT, rhs, **kw)


def build_mha(ctx: ExitStack, tc: tile.TileContext, y, xT, wqk, wv, wo):
    nc = tc.nc

    persist = ctx.enter_context(tc.tile_pool(name="persist", bufs=1))

    # Persistent SBUF tensors
    qT = [persist.tile([128, S], MM_DT, tag=f"qT{g}", name=f"qT{g}") for g in range(NGROUPS)]
    kT = [persist.tile([128, S], MM_DT, tag=f"kT{g}", name=f"kT{g}") for g in range(NGROUPS)]
    # Vaug[h][t]: [V_h tile (64 cols) | ones col] -> M=65 attn@V matmul computes
    # the denominator in the same pass
    Vaug = [[persist.tile([128, DK + 1], MM_DT, tag=f"Va{h}_{t}", name=f"Va{h}_{t}")
             for t in range(NT)] for h in range(HPC)]
    outT = [persist.tile([64, S], MM_DT, tag=f"outT{h}", name=f"outT{h}") for h in range(HPC)]
    wo_sb = [persist.tile([64, D], MM_DT, tag=f"wo{h}", name=f"wo{h}") for h in range(HPC)]
    ones32 = persist.tile([128, 1], F32, tag="ones32")
    nc.vector.memset(ones32[:], 1.0)
    for h in range(HPC):
        nc.sync.dma_start(out=wo_sb[h][:], in_=wo[h * DK:(h + 1) * DK, :])
        for t in range(NT):
            nc.vector.tensor_copy(Vaug[h][t][:, DK:DK + 1], ones32[:])

    # ---------------- Phase 1: projections ----------------
    with (
        tc.tile_pool(name="p1_in", bufs=1) as p1_in,
        tc.tile_pool(name="p1_psqk", bufs=2, space="PSUM") as psqk_pool,
        tc.tile_pool(name="p1_psv", bufs=2, space="PSUM") as psv_pool,
    ):
        xT_sb = [p1_in.tile([128, S], MM_DT, tag=f"xT{c}", name=f"xTs{c}") for c in range(NCH)]
        wqk_sb = [p1_in.tile([128, 2 * HPC * DK], MM_DT, tag=f"wqk{c}", name=f"wqks{c}") for c in range(NCH)]
        wv_sb = [p1_in.tile([128, HPC * DK], MM_DT, tag=f"wv{c}", name=f"wvs{c}") for c in range(NCH)]
        for c in range(NCH):
            nc.sync.dma_start(out=xT_sb[c][:], in_=xT[c * 128:(c + 1) * 128, :])
            nc.sync.dma_start(out=wqk_sb[c][:], in_=wqk[c * 128:(c + 1) * 128, :])
            nc.sync.dma_start(out=wv_sb[c][:], in_=wv[c * 128:(c + 1) * 128, :])

        # qT/kT: psum[col=128, tok=512] = sum_ch wqk[ch, col].T @ xT[ch, tok]
        # col-tile ct: 0 -> qT[0], 1 -> qT[1], 2 -> kT[0], 3 -> kT[1]
        dests = [qT[0], qT[1], kT[0], kT[1]]
        for ct in range(4):
            for sc in range(NSC):
                ps = psqk_pool.tile([128, SC], F32, tag="psqk")
                for c in range(NCH):
                    _mm(nc, ps[:], wqk_sb[c][:, ct * 128:(ct + 1) * 128],
                        xT_sb[c][:, sc * SC:(sc + 1) * SC],
                        start=(c == 0), stop=(c == NCH - 1))
                nc.vector.tensor_copy(dests[ct][:, sc * SC:(sc + 1) * SC], ps[:])

        # V: psum[tok=128, vcol=256] = sum_ch xT[ch, tok].T @ wv[ch, :]
        for t in range(NT):
            ps = psv_pool.tile([128, HPC * DK], F32, tag="psv")
            for c in range(NCH):
                _mm(nc, ps[:], xT_sb[c][:, t * 128:(t + 1) * 128], wv_sb[c][:],
                    start=(c == 0), stop=(c == NCH - 1))
            for h in range(HPC):
                nc.vector.tensor_copy(Vaug[h][t][:, 0:DK], ps[:, h * DK:(h + 1) * DK])

    # ---------------- Phase 2: attention ----------------
    with (
        tc.tile_pool(name="p2_ps", bufs=2, space="PSUM") as score_pool,
        tc.tile_pool(name="p2_u", bufs=2, space="PSUM") as u_pool,
        tc.tile_pool(name="p2_exp", bufs=3) as exp_pool,
        tc.tile_pool(name="p2_rb", bufs=2) as rb_pool,
    ):
        def _bcast_row(row_ap):
            # read the single SBUF row 64x via a step-0 free dim
            return bass.AP(tensor=row_ap.tensor, offset=row_ap.offset,
                           ap=[row_ap.ap[0], [0, 64], row_ap.ap[1]])

        for g in range(NGROUPS):
            hA, hB = 2 * g, 2 * g + 1
            for sc in range(NSC):
                qs = slice(sc * SC, (sc + 1) * SC)
                psu_a = u_pool.tile([128, SC], F32, tag="psu_a")
                psu_b = u_pool.tile([128, SC], F32, tag="psu_b")
                for t in range(NT):
                    ts = slice(t * 128, (t + 1) * 128)
                    ps_a = score_pool.tile([128, SC], F32, tag="ps_a")
                    ps_b = score_pool.tile([128, SC], F32, tag="ps_b")
                    # scoresT[t, s] for the two heads (K=64 row-packed)
                    _mm(nc, ps_a[:], kT[g][0:64, ts], qT[g][0:64, qs],
                        tile_position=(0, 0), start=True, stop=True)
                    _mm(nc, ps_b[:], kT[g][64:128, ts], qT[g][64:128, qs],
                        tile_position=(64, 0), start=True, stop=True)
                    e_a = exp_pool.tile([128, SC], MM_DT, tag="e_a")
                    e_b = exp_pool.tile([128, SC], MM_DT, tag="e_b")
                    nc.scalar.activation(e_a[:], ps_a[:],
                                         mybir.ActivationFunctionType.Exp, scale=SCALE)
                    nc.scalar.activation(e_b[:], ps_b[:],
                                         mybir.ActivationFunctionType.Exp, scale=SCALE)
                    # [attn@V | den] via M=65 [V|ones] matmuls, one per head
                    st, sp = (t == 0), (t == NT - 1)
                    _mm(nc, psu_a[0:DK + 1, :], Vaug[hA][t][:], e_a[:],
                        start=st, stop=sp, skip_group_check=True)
                    _mm(nc, psu_b[0:DK + 1, :], Vaug[hB][t][:], e_b[:],
                        start=st, stop=sp, skip_group_check=True)
                # normalize: reciprocal psum->sbuf, broadcast across partitions, multiply
                dsb = rb_pool.tile([128, 2 * SC], F32, tag="dsb")
                nc.vector.reciprocal(dsb[DK:DK + 1, 0:SC], psu_a[DK:DK + 1, :])
                nc.vector.reciprocal(dsb[DK:DK + 1, SC:2 * SC], psu_b[DK:DK + 1, :])
                rb_a = rb_pool.tile([64, SC], F32, tag="rb_a")
                rb_b = rb_pool.tile([64, SC], F32, tag="rb_b")
                nc.sync.dma_start(out=rb_a[:], in_=_bcast_row(dsb[DK:DK + 1, 0:SC]))
                nc.sync.dma_start(out=rb_b[:], in_=_bcast_row(dsb[DK:DK + 1, SC:2 * SC]))
                nc.vector.tensor_mul(outT[hA][:, qs], psu_a[0:DK, :], rb_a[:])
                nc.vector.tensor_mul(outT[hB][:, qs], psu_b[0:DK, :], rb_b[:])

    # ---------------- Phase 3: output projection ----------------
    with (
        tc.tile_pool(name="p3_ps", bufs=4, space="PSUM") as y_pool,
        tc.tile_pool(name="p3_sb", bufs=4) as ysb_pool,
    ):
        for st in range(NT):
            for dc in range(D // SC):
                ps = y_pool.tile([128, SC], F32, tag="psy")
                for h in range(HPC):
                    _mm(nc, ps[:], outT[h][:, st * 128:(st + 1) * 128],
                        wo_sb[h][:, dc * SC:(dc + 1) * SC],
                        start=(h == 0), stop=(h == HPC - 1))
                ysb = ysb_pool.tile([128, SC], F32, tag="ysb")
                nc.vector.tensor_copy(ysb[:], ps[:])
                nc.sync.dma_start(out=y[st * 128:(st + 1) * 128, dc * SC:(dc + 1) * SC],
                                  in_=ysb[:])


_CACHED_NC = None


def _build_nc():
    global _CACHED_NC
    if _CACHED_NC is not None:
        return _CACHED_NC
    nc = bacc.Bacc("TRN2", target_bir_lowering=False, debug=False,
                   num_devices=N_CORES)
    xT = nc.dram_tensor("xT", [D, S], MM_DT, kind="ExternalInput").ap()
    wqk = nc.dram_tensor("wqk", [D, 2 * HPC * DK], MM_DT, kind="ExternalInput").ap()
    wv = nc.dram_tensor("wv", [D, HPC * DK], MM_DT, kind="ExternalInput").ap()
    wo = nc.dram_tensor("wo", [HPC * DK, D], MM_DT, kind="ExternalInput").ap()
    y = nc.dram_tensor("y", [S, D], F32, kind="ExternalOutput").ap()
    with tile.TileContext(nc) as tc:
        with ExitStack() as ctx:
            build_mha(ctx, tc, y, xT, wqk, wv, wo)
    nc.compile()
    _CACHED_NC = nc
    return nc


def make_in_maps(x, w_qkv, w_o):
    """Shard the full inputs into the 8 per-core input dicts."""
    x = np.asarray(x, dtype=np.float32)
    w_qkv = np.asarray(w_qkv, dtype=np.float32)
    w_o = np.asarray(w_o, dtype=np.float32)
    in_maps = []
    for c in range(N_CORES):
        b, hb = c // HPC, c % HPC
        heads = [HPC * hb + i for i in range(HPC)]
        q_cols = [w_qkv[:, h * DK:(h + 1) * DK] for h in heads]
        k_cols = [w_qkv[:, D + h * DK:D + (h + 1) * DK] for h in heads]
        v_cols = [w_qkv[:, 2 * D + h * DK:2 * D + (h + 1) * DK] for h in heads]
        in_maps.append({
            "xT": np.ascontiguousarray(x[b].T),
            "wqk": np.ascontiguousarray(np.concatenate(q_cols + k_cols, axis=1)),
            "wv": np.ascontiguousarray(np.concatenate(v_cols, axis=1)),
            "wo": np.ascontiguousarray(w_o[HPC * hb * DK:HPC * hb * DK + HPC * DK, :]),
        })
    return in_maps


_LAST_RESULT = None  # BassKernelResults of the most recent run (for profiling)


def kernel(x, w_qkv, w_o, **run_kwargs):
    global _LAST_RESULT
    nc = _build_nc()
    in_maps = make_in_maps(x, w_qkv, w_o)
    try:
        res = run_bass_kernel_spmd(nc, in_maps, core_ids=list(range(N_CORES)),
                                   **run_kwargs)
    except ModuleNotFoundError:
        # NTFF trace hook unavailable in this container: rerun untraced
        os.environ["BASS_NEVER_TRACE"] = "1"
        res = run_bass_kernel_spmd(nc, in_maps, core_ids=list(range(N_CORES)))
    _LAST_RESULT = res
    out = np.zeros((B, S, D), dtype=np.float32)
    for c in range(N_CORES):
        out[c // HPC] += res.results[c]["y"]
    return out


# revision 14
# speedup vs baseline: 1.0889x; 1.0889x over previous
"""Multi-head attention on 8 TRN2 NeuronCores.

Problem: x[2, 2048, 1024] @ w_qkv[1024, 3072] -> 16-head attention -> @ w_o[1024, 1024].

Sharding: core c handles batch b = c//4 and 4 heads [4*(c%4), 4*(c%4)+4).
Each core computes a full partial output y_c[2048, 1024] = attn_out_heads @ w_o_rows;
host sums the 4 partials per batch (the "all-reduce" of the row-split w_o).

Per-core layouts (host-prepped so the kernel never transposes on-chip):
  xT  [1024, 2048]  = x[b].T
  wqk [1024, 512]   cols = q(h0),q(h1),q(h2),q(h3),k(h0)..k(h3)  (64 each)
  wv  [1024, 256]   cols = v(h0)..v(h3)
  wo  [256, 1024]   rows = w_o rows for the 4 heads
Phase 1 (projections):
  qT/kT via weights-stationary matmuls -> [head-dim rows, tokens] directly
  V    via xT-stationary matmuls       -> [tokens, head cols] directly
Phase 2 (attention, per 2-head group, per 512-query chunk, streaming over 16 key tiles):
  scoresT[t,s] psum = kT.T @ qT   (two K=64 matmuls row-packed at tile_position (0,0)/(64,0))
  expT = exp(scoresT * 0.125) on ACT (scores ~ N(0,1): no max subtraction needed)
  [uout | den] += [V_tile | ones].T @ expT   (M=65 matmul per head: rows 0-63 are
      the attention output, row 64 is the softmax denominator; fp32r cannot
      col-tile on this walrus, so the two heads are sequential streams)
  normalize: reciprocal(den) -> DMA partition-broadcast -> multiply -> outT_h[d,s]
Phase 3: y[s, :] = sum_h outT_h.T @ wo_h (K=64 per head), DMA via SBUF to DRAM.
"""

import os
from contextlib import ExitStack

import numpy as np

import concourse.bass as bass
import concourse.tile as tile
from concourse import bacc, mybir
from concourse.bass_utils import run_bass_kernel_spmd

F32 = mybir.dt.float32
# float32r: full-rate (1 cyc/row) PE matmuls on fp32 data with slightly
# reduced multiply precision (the BIR verifier requires every fp32r matmul
# operand to be *produced* as fp32r, so the whole data chain up to the PE is
# declared float32r; its numpy binding is plain float32). Set BASS_MHA_FP32=1
# to fall back to exact (4x slower) fp32 matmuls.
MM_DT = F32 if os.environ.get("BASS_MHA_FP32") == "1" else mybir.dt.float32r

B, S, D = 2, 2048, 1024
H, DK = 16, 64
N_CORES = 8
HPC = 4           # heads per core
NGROUPS = 2       # head groups per core (2 heads each)
SC = 512          # query-chunk (matmul streaming N)
NSC = S // SC     # 4 query chunks
NT = S // 128     # 16 key tiles
NCH = D // 128    # 8 contraction tiles for the projections
SCALE = DK ** -0.5


def _mm(nc, out, lhsT, rhs, **kw):
    nc.tensor.matmul(out, lhsT, rhs, **kw)


def build_mha(ctx: ExitStack, tc: tile.TileContext, y, xT, wqk, wv, wo):
    nc = tc.nc

    persist = ctx.enter_context(tc.tile_pool(name="persist", bufs=1))

    # Persistent SBUF tensors
    qT = [persist.tile([128, S], MM_DT, tag=f"qT{g}", name=f"qT{g}") for g in range(NGROUPS)]
    kT = [persist.tile([128, S], MM_DT, tag=f"kT{g}", name=f"kT{g}") for g in range(NGROUPS)]
    # Vaug[h][t]: [V_h tile (64 cols) | ones col] -> M=65 attn@V matmul computes
    # the denominator in the same pass
    Vaug = [[persist.tile([128, DK + 1], MM_DT, tag=f"Va{h}_{t}", name=f"Va{h}_{t}")
             for t in range(NT)] for h in range(HPC)]
    # outT[h][sc]: per query-chunk tiles so the output projection of chunk sc
    # can start while later chunks are still in the attention loop
    outT = [[persist.tile([64, SC], MM_DT, tag=f"oT{h}_{sc}", name=f"oT{h}_{sc}")
             for sc in range(NSC)] for h in range(HPC)]
    wo_sb = [persist.tile([64, D], MM_DT, tag=f"wo{h}", name=f"wo{h}") for h in range(HPC)]
    ones32 = persist.tile([128, 1], F32, tag="ones32")
    nc.vector.memset(ones32[:], 1.0)
    for h in range(HPC):
        for t in range(NT):
            nc.vector.tensor_copy(Vaug[h][t][:, DK:DK + 1], ones32[:])

    # ---------------- Phase 1: projections ----------------
    with (
        tc.tile_pool(name="p1_in", bufs=1) as p1_in,
        tc.tile_pool(name="p1_psqk", bufs=3, space="PSUM") as psqk_pool,
        tc.tile_pool(name="p1_psv", bufs=3, space="PSUM") as psv_pool,
    ):
        xT_sb = [p1_in.tile([128, S], MM_DT, tag=f"xT{c}", name=f"xTs{c}") for c in range(NCH)]
        wqk_sb = [p1_in.tile([128, 2 * HPC * DK], MM_DT, tag=f"wqk{c}", name=f"wqks{c}") for c in range(NCH)]
        wv_sb = [p1_in.tile([128, HPC * DK], MM_DT, tag=f"wv{c}", name=f"wvs{c}") for c in range(NCH)]
        # arrival order matches first consumption: wqk + xT chunk-by-chunk,
        # then wv (V runs after ct0/ct2), wo last (only phase 3 needs it)
        for c in range(NCH):
            nc.sync.dma_start(out=wqk_sb[c][:], in_=wqk[c * 128:(c + 1) * 128, :])
            nc.sync.dma_start(out=xT_sb[c][:, 0:SC], in_=xT[c * 128:(c + 1) * 128, 0:SC])
        for sc in range(1, NSC):
            for c in range(NCH):
                nc.sync.dma_start(out=xT_sb[c][:, sc * SC:(sc + 1) * SC],
                                  in_=xT[c * 128:(c + 1) * 128, sc * SC:(sc + 1) * SC])
        for c in range(NCH):
            nc.sync.dma_start(out=wv_sb[c][:], in_=wv[c * 128:(c + 1) * 128, :])
        for h in range(HPC):
            nc.sync.dma_start(out=wo_sb[h][:], in_=wo[h * DK:(h + 1) * DK, :])

        # qT/kT: psum[col=128, tok=512] = sum_ch wqk[ch, col].T @ xT[ch, tok]
        # col-tile ct: 0 -> qT[0], 1 -> qT[1], 2 -> kT[0], 3 -> kT[1]
        # All of phase 1 is interleaved per query chunk so PE work tracks the
        # chunked xT arrivals instead of serializing behind the full load.
        dests = {0: qT[0], 1: qT[1], 2: kT[0], 3: kT[1]}
        for sc in range(NSC):
            # group-1 col-tiles first: the last psum-bank users in phase 1 are
            # then exactly the tensors phase 2's first group waits on anyway
            for ct in (1, 3, 0, 2):
                ps = psqk_pool.tile([128, SC], F32, tag="psqk")
                for c in range(NCH):
                    _mm(nc, ps[:], wqk_sb[c][:, ct * 128:(ct + 1) * 128],
                        xT_sb[c][:, sc * SC:(sc + 1) * SC],
                        start=(c == 0), stop=(c == NCH - 1))
                # alternate psum evacuation between DVE and the (idle) ACT engine
                if ct in (0, 1):
                    nc.vector.tensor_copy(dests[ct][:, sc * SC:(sc + 1) * SC], ps[:])
                else:
                    nc.scalar.copy(dests[ct][:, sc * SC:(sc + 1) * SC], ps[:])
            # V: psum[tok=128, vcol=256] = sum_ch xT[ch, tok].T @ wv[ch, :]
            for t in range(4 * sc, 4 * sc + 4):
                ps = psv_pool.tile([128, HPC * DK], F32, tag="psv")
                for c in range(NCH):
                    _mm(nc, ps[:], xT_sb[c][:, t * 128:(t + 1) * 128], wv_sb[c][:],
                        start=(c == 0), stop=(c == NCH - 1))
                for h in range(HPC):
                    if h % 2 == 0:
                        nc.vector.tensor_copy(Vaug[h][t][:, 0:DK], ps[:, h * DK:(h + 1) * DK])
                    else:
                        nc.scalar.copy(Vaug[h][t][:, 0:DK], ps[:, h * DK:(h + 1) * DK])

    # ---------------- Phase 2 + 3 interleaved per query chunk ----------------
    with (
        tc.tile_pool(name="p2_ps", bufs=2, space="PSUM") as score_pool,
        tc.tile_pool(name="p2_u", bufs=1, space="PSUM") as u_pool,
        tc.tile_pool(name="p2_exp", bufs=3) as exp_pool,
        tc.tile_pool(name="p2_rb", bufs=2) as rb_pool,
        tc.tile_pool(name="p3_ps", bufs=2, space="PSUM") as y_pool,
        tc.tile_pool(name="p3_sb", bufs=3) as ysb_pool,
    ):
        def _emit_proj(psc):
            for st4 in range(SC // 128):
                srow = psc * SC + st4 * 128
                for dc in range(D // SC):
                    ps = y_pool.tile([128, SC], F32, tag="psy")
                    for h in range(HPC):
                        _mm(nc, ps[:], outT[h][psc][:, st4 * 128:(st4 + 1) * 128],
                            wo_sb[h][:, dc * SC:(dc + 1) * SC],
                            start=(h == 0), stop=(h == HPC - 1))
                    ysb = ysb_pool.tile([128, SC], F32, tag="ysb")
                    nc.vector.tensor_copy(ysb[:], ps[:])
                    nc.sync.dma_start(out=y[srow:srow + 128, dc * SC:(dc + 1) * SC],
                                      in_=ysb[:])

        def _bcast_row(row_ap):
            # read the single SBUF row 64x via a step-0 free dim
            return bass.AP(tensor=row_ap.tensor, offset=row_ap.offset,
                           ap=[row_ap.ap[0], [0, 64], row_ap.ap[1]])

        for sc in range(NSC):
            qs = slice(sc * SC, (sc + 1) * SC)
            for g in range(NGROUPS):
                hA, hB = 2 * g, 2 * g + 1
                psu_a = u_pool.tile([128, SC], F32, tag="psu_a")
                psu_b = u_pool.tile([128, SC], F32, tag="psu_b")
                for t in range(NT):
                    ts = slice(t * 128, (t + 1) * 128)
                    # scoresT[t, s] for both heads into one 2-bank psum tile
                    ps = score_pool.tile([128, 2 * SC], F32, tag="ps")
                    _mm(nc, ps[:, 0:SC], kT[g][0:64, ts], qT[g][0:64, qs],
                        tile_position=(0, 0), start=True, stop=True)
                    _mm(nc, ps[:, SC:2 * SC], kT[g][64:128, ts], qT[g][64:128, qs],
                        tile_position=(64, 0), start=True, stop=True)
                    e = exp_pool.tile([128, 2 * SC], MM_DT, tag="e")
                    nc.scalar.activation(e[:], ps[:],
                                         mybir.ActivationFunctionType.Exp, scale=SCALE)
                    # [attn@V | den] via M=65 [V|ones] matmuls, one per head
                    st, sp = (t == 0), (t == NT - 1)
                    _mm(nc, psu_a[0:DK + 1, :], Vaug[hA][t][:], e[:, 0:SC],
                        start=st, stop=sp, skip_group_check=True)
                    _mm(nc, psu_b[0:DK + 1, :], Vaug[hB][t][:], e[:, SC:2 * SC],
                        start=st, stop=sp, skip_group_check=True)
                # evacuate psum immediately (frees the accumulator bank for the
                # next head group), then normalize from SBUF
                usb_a = rb_pool.tile([DK + 1, SC], F32, tag="usb_a")
                usb_b = rb_pool.tile([DK + 1, SC], F32, tag="usb_b")
                nc.vector.tensor_copy(usb_a[:], psu_a[0:DK + 1, :])
                nc.vector.tensor_copy(usb_b[:], psu_b[0:DK + 1, :])
                dsb = rb_pool.tile([128, 2 * SC], F32, tag="dsb")
                nc.vector.reciprocal(dsb[DK:DK + 1, 0:SC], usb_a[DK:DK + 1, :])
                nc.vector.reciprocal(dsb[DK:DK + 1, SC:2 * SC], usb_b[DK:DK + 1, :])
                rb_a = rb_pool.tile([64, SC], F32, tag="rb_a")
                rb_b = rb_pool.tile([64, SC], F32, tag="rb_b")
                nc.sync.dma_start(out=rb_a[:], in_=_bcast_row(dsb[DK:DK + 1, 0:SC]))
                nc.sync.dma_start(out=rb_b[:], in_=_bcast_row(dsb[DK:DK + 1, SC:2 * SC]))
                nc.vector.tensor_mul(outT[hA][sc][:], usb_a[0:DK, :], rb_a[:])
                nc.vector.tensor_mul(outT[hB][sc][:], usb_b[0:DK, :], rb_b[:])

            # output projection, deferred one chunk: emitted after the NEXT
            # chunk's attention has priority, so these matmuls gap-fill the
            # ACT-bound attention loop instead of stalling it
            for psc in ([sc - 1] if sc >= 1 else []) + ([sc] if sc == NSC - 1 else []):
                _emit_proj(psc)


_CACHED_NC = None


def _build_nc():
    global _CACHED_NC
    if _CACHED_NC is not None:
        return _CACHED_NC
    nc = bacc.Bacc("TRN2", target_bir_lowering=False, debug=False,
                   num_devices=N_CORES)
    xT = nc.dram_tensor("xT", [D, S], MM_DT, kind="ExternalInput").ap()
    wqk = nc.dram_tensor("wqk", [D, 2 * HPC * DK], MM_DT, kind="ExternalInput").ap()
    wv = nc.dram_tensor("wv", [D, HPC * DK], MM_DT, kind="ExternalInput").ap()
    wo = nc.dram_tensor("wo", [HPC * DK, D], MM_DT, kind="ExternalInput").ap()
    y = nc.dram_tensor("y", [S, D], F32, kind="ExternalOutput").ap()
    with tile.TileContext(nc) as tc:
        with ExitStack() as ctx:
            build_mha(ctx, tc, y, xT, wqk, wv, wo)
    nc.compile()
    _CACHED_NC = nc
    return nc


def make_in_maps(x, w_qkv, w_o):
    """Shard the full inputs into the 8 per-core input dicts."""
    x = np.asarray(x, dtype=np.float32)
    w_qkv = np.asarray(w_qkv, dtype=np.float32)
    w_o = np.asarray(w_o, dtype=np.float32)
    in_maps = []
    for c in range(N_CORES):
        b, hb = c // HPC, c % HPC
        heads = [HPC * hb + i for i in range(HPC)]
        q_cols = [w_qkv[:, h * DK:(h + 1) * DK] for h in heads]
        k_cols = [w_qkv[:, D + h * DK:D + (h + 1) * DK] for h in heads]
        v_cols = [w_qkv[:, 2 * D + h * DK:2 * D + (h + 1) * DK] for h in heads]
        in_maps.append({
            "xT": np.ascontiguousarray(x[b].T),
            "wqk": np.ascontiguousarray(np.concatenate(q_cols + k_cols, axis=1)),
            "wv": np.ascontiguousarray(np.concatenate(v_cols, axis=1)),
            "wo": np.ascontiguousarray(w_o[HPC * hb * DK:HPC * hb * DK + HPC * DK, :]),
        })
    return in_maps


_LAST_RESULT = None  # BassKernelResults of the most recent run (for profiling)


def kernel(x, w_qkv, w_o, **run_kwargs):
    global _LAST_RESULT
    nc = _build_nc()
    in_maps = make_in_maps(x, w_qkv, w_o)
    try:
        res = run_bass_kernel_spmd(nc, in_maps, core_ids=list(range(N_CORES)),
                                   **run_kwargs)
    except ModuleNotFoundError:
        # NTFF trace hook unavailable in this container: rerun untraced
        os.environ["BASS_NEVER_TRACE"] = "1"
        res = run_bass_kernel_spmd(nc, in_maps, core_ids=list(range(N_CORES)))
    _LAST_RESULT = res
    out = np.zeros((B, S, D), dtype=np.float32)
    for c in range(N_CORES):
        out[c // HPC] += res.results[c]["y"]
    return out
